# revision 3
# baseline (speedup 1.0000x reference)
"""AttentionSuper (AutoFormer relative-position attention) on 8 trn2 cores.

Strategy: data-parallel over batch B=64 -> 8 batches/core. Per core, attention
is computed in TRANSPOSED score layout attnT[j, i] per (b,h) so that:
  - the value matmuls (attn @ v, attn @ rel_v) need attnT as lhsT directly
    (no on-chip transposes), and
  - softmax normalization is obtained free via a ones-column appended to v
    (sum over j = partition axis comes out of the same matmul).
Scores are bounded (|scaled score| < ~6) so softmax skips max-subtraction.
The rel-pos bias biasT[j, i] (= q_i . rel_k[i,j]) is computed per-i as a
matmul rel_kT_i.T @ q_iT over all (b,h) at once and kept resident in SBUF
(bf16). rel_k / rel_v are materialized on host from the 30-row tables (pure
index gather, part of input prep).
"""

import sys

import numpy as np

sys.path.insert(0, "/opt/trn_rl_repo")

import ml_dtypes  # noqa: E402

B, N, H, D = 64, 197, 10, 64
MAX_REL = 14
NCORES = 8
BSH = B // NCORES          # batches per core
BH = BSH * H               # 80 fused (batch, head) rows per core
P1, P2 = 128, N - 128      # 128 + 69 partition split of N
SCALE = D ** (-0.5)

_bf16 = ml_dtypes.bfloat16

LAST_EXEC_NS = None
LAST_TRACE = None
_CACHED = None


def _rel_indices():
    s = int(np.sqrt(N))
    r = np.arange(N)
    dist_v = r[None, :] // s - r[:, None] // s
    dist_h = r[None, :] % s - r[:, None] % s
    iv = np.clip(dist_v, -MAX_REL, MAX_REL) + MAX_REL + 1
    ih = np.clip(dist_h, -MAX_REL, MAX_REL) + MAX_REL + 1
    iv = np.pad(iv[1:, 1:], ((1, 0), (1, 0)), constant_values=0)
    ih = np.pad(ih[1:, 1:], ((1, 0), (1, 0)), constant_values=0)
    return iv, ih


def _build_module():
    import concourse.bass as bass  # noqa: F401
    import concourse.bacc as bacc
    import concourse.tile as tile
    from concourse import mybir

    f32 = mybir.dt.float32
    bf16 = mybir.dt.bfloat16
    Exp = mybir.ActivationFunctionType.Exp

    nc = bacc.Bacc()

    qT = nc.dram_tensor("qT", [BH, D, N], f32, kind="ExternalInput")
    kT = nc.dram_tensor("kT", [BH, D, N], f32, kind="ExternalInput")
    vb = nc.dram_tensor("vb", [BH, N, D], bf16, kind="ExternalInput")
    qTi = nc.dram_tensor("qTi", [D, N, BH], bf16, kind="ExternalInput")
    rkT = nc.dram_tensor("rkT", [D, N, N], bf16, kind="ExternalInput")
    rv = nc.dram_tensor("rv", [N, N, D], bf16, kind="ExternalInput")
    out = nc.dram_tensor("out", [BSH, N, H * D], f32, kind="ExternalOutput")
    o2d = nc.dram_tensor("o2d", [BH, N, D], f32)  # internal: rel-v partial

    with tile.TileContext(nc) as tc:
        with tc.tile_pool(name="persist", bufs=1) as persist:
            biasT_lo = persist.tile([128, N, BH], bf16)   # j in [0,128)
            biasT_hi = persist.tile([128, N, BH], bf16)   # j in [128,197) on parts 0..68
            attnT_lo = persist.tile([128, BH, N], bf16)
            attnT_hi = persist.tile([128, BH, N], bf16)
            out_lo = persist.tile([128, BH, D], f32)      # i in [0,128)
            out_hi = persist.tile([128, BH, D], f32)      # i in [128,197)
            recips_lo = persist.tile([128, BH], f32)
            recips_hi = persist.tile([128, BH], f32)

            # ---- Phase A: biasT[j, i*BH+bh] = sum_d rel_k[i,j,d] q[bh,i,d]
            CH = 16
            G = 4
            with (
                tc.tile_pool(name="pa", bufs=2) as pa,
                tc.tile_pool(name="pap", bufs=3, space="PSUM") as pap,
            ):
                for c0 in range(0, N, CH):
                    cn = min(CH, N - c0)
                    rk_t = pa.tile([D, CH, N], bf16, tag="rk")
                    nc.sync.dma_start(rk_t[:, :cn, :], rkT[:, c0 : c0 + cn, :])
                    qb_t = pa.tile([D, CH, BH], bf16, tag="qb")
                    nc.sync.dma_start(qb_t[:, :cn, :], qTi[:, c0 : c0 + cn, :])
                    for g0 in range(0, cn, G):
                        gn = min(G, cn - g0)
                        ps_lo = pap.tile([128, G * BH], f32, tag="pslo")
                        ps_hi = pap.tile([128, G * BH], f32, tag="pshi")
                        for ii in range(gn):
                            nc.tensor.matmul(
                                ps_lo[:, ii * BH : (ii + 1) * BH],
                                rk_t[:, g0 + ii, 0:128],
                                qb_t[:, g0 + ii, :],
                                start=True, stop=True,
                            )
                            nc.tensor.matmul(
                                ps_hi[0:P2, ii * BH : (ii + 1) * BH],
                                rk_t[:, g0 + ii, 128:N],
                                qb_t[:, g0 + ii, :],
                                start=True, stop=True,
                            )
                        i0 = c0 + g0
                        nc.vector.tensor_copy(
                            biasT_lo[:, i0 : i0 + gn, :], ps_lo[:, : gn * BH]
                        )
                        nc.vector.tensor_copy(
                            biasT_hi[0:P2, i0 : i0 + gn, :], ps_hi[0:P2, : gn * BH]
                        )

            # ---- Phase B1: per (b,h): scores^T, +bias, exp, O1 = attnT.T @ [v|1]
            with (
                tc.tile_pool(name="pb", bufs=3) as pb,
                tc.tile_pool(name="pbp", bufs=2, space="PSUM") as pbp,
                tc.tile_pool(name="pbp2", bufs=2, space="PSUM") as pbp2,
            ):
                for bh in range(BH):
                    qt = pb.tile([D, N], f32, tag="qt")
                    nc.sync.dma_start(qt[:], qT[bh])
                    kt = pb.tile([D, N], f32, tag="kt")
                    nc.sync.dma_start(kt[:], kT[bh])
                    vlo = pb.tile([128, D + 1], bf16, tag="vlo")
                    nc.sync.dma_start(vlo[:, 0:D], vb[bh, 0:128, :])
                    nc.vector.memset(vlo[:, D : D + 1], 1.0)
                    vhi = pb.tile([128, D + 1], bf16, tag="vhi")
                    nc.sync.dma_start(vhi[0:P2, 0:D], vb[bh, 128:N, :])
                    nc.vector.memset(vhi[0:P2, D : D + 1], 1.0)

                    slo = pbp.tile([128, N], f32, tag="slo")
                    nc.tensor.matmul(slo[:], kt[:, 0:128], qt[:], start=True, stop=True)
                    shi = pbp.tile([128, N], f32, tag="shi")
                    nc.tensor.matmul(
                        shi[0:P2, :], kt[:, 128:N], qt[:], start=True, stop=True
                    )

                    tlo = pb.tile([128, N], f32, tag="tlo")
                    nc.vector.tensor_add(tlo[:], slo[:], biasT_lo[:, :, bh])
                    nc.scalar.activation(
                        attnT_lo[:, bh, :], tlo[:], Exp, scale=SCALE
                    )
                    thi = pb.tile([128, N], f32, tag="thi")
                    nc.vector.tensor_add(
                        thi[0:P2, :], shi[0:P2, :], biasT_hi[0:P2, :, bh]
                    )
                    nc.scalar.activation(
                        attnT_hi[0:P2, bh, :], thi[0:P2, :], Exp, scale=SCALE
                    )

                    for c0, cn, o_t, r_t in (
                        (0, 128, out_lo, recips_lo),
                        (128, P2, out_hi, recips_hi),
                    ):
                        o1 = pbp2.tile([128, D + 1], f32, tag="o1")
                        nc.tensor.matmul(
                            o1[0:cn, :], attnT_lo[:, bh, c0 : c0 + cn], vlo[:, :],
                            start=True, stop=False,
                        )
                        nc.tensor.matmul(
                            o1[0:cn, :], attnT_hi[0:P2, bh, c0 : c0 + cn],
                            vhi[0:P2, :], start=False, stop=True,
                        )
                        nc.vector.reciprocal(
                            r_t[0:cn, bh : bh + 1], o1[0:cn, D : D + 1]
                        )
                        nc.vector.tensor_scalar_mul(
                            o_t[0:cn, bh, :], o1[0:cn, 0:D], r_t[0:cn, bh : bh + 1]
                        )

            # ---- Phase B2: O2[bh, i, d] = sum_j attnT[j, i] rel_v[i, j, d]
            CH2 = 8
            G2 = 4
            with (
                tc.tile_pool(name="pc", bufs=3) as pc,
                tc.tile_pool(name="pcp", bufs=3, space="PSUM") as pcp,
            ):
                for c0 in range(0, N, CH2):
                    cn = min(CH2, N - c0)
                    rvl = pc.tile([128, CH2, D], bf16, tag="rvl")
                    nc.sync.dma_start(rvl[:, :cn, :], rv[0:128, c0 : c0 + cn, :])
                    rvh = pc.tile([128, CH2, D], bf16, tag="rvh")
                    nc.sync.dma_start(rvh[0:P2, :cn, :], rv[128:N, c0 : c0 + cn, :])
                    for g0 in range(0, cn, G2):
                        gn = min(G2, cn - g0)
                        o2 = pcp.tile([BH, G2 * D], f32, tag="o2")
                        for ii in range(gn):
                            i = c0 + g0 + ii
                            nc.tensor.matmul(
                                o2[:, ii * D : (ii + 1) * D],
                                attnT_lo[:, :, i], rvl[:, g0 + ii, :],
                                start=True, stop=False,
                            )
                            nc.tensor.matmul(
                                o2[:, ii * D : (ii + 1) * D],
                                attnT_hi[0:P2, :, i], rvh[0:P2, g0 + ii, :],
                                start=False, stop=True,
                            )
                        i0 = c0 + g0
                        o2s = pc.tile([BH, G2 * D], f32, tag="o2s")
                        nc.vector.tensor_copy(o2s[:, : gn * D], o2[:, : gn * D])
                        nc.sync.dma_start(
                            o2d[:, i0 : i0 + gn, :], o2s[:, : gn * D]
                        )

            tc.strict_bb_all_engine_barrier()

            # ---- Phase C: out[b, i, h*D:] = O1(normalized) + O2 * recip
            with tc.tile_pool(name="pd", bufs=4) as pd:
                for bh in range(BH):
                    b, h = divmod(bh, H)
                    for c0, cn, o_t, r_t in (
                        (0, 128, out_lo, recips_lo),
                        (128, P2, out_hi, recips_hi),
                    ):
                        t2 = pd.tile([128, D], f32, tag="t2")
                        nc.sync.dma_start(t2[0:cn, :], o2d[bh, c0 : c0 + cn, :])
                        t3 = pd.tile([128, D], f32, tag="t3")
                        nc.vector.tensor_scalar_mul(
                            t3[0:cn, :], t2[0:cn, :], r_t[0:cn, bh : bh + 1]
                        )
                        res = pd.tile([128, D], f32, tag="res")
                        nc.vector.tensor_add(
                            res[0:cn, :], t3[0:cn, :], o_t[0:cn, bh, :]
                        )
                        nc.sync.dma_start(
                            out[b, c0 : c0 + cn, h * D : (h + 1) * D], res[0:cn, :]
                        )

    nc.finalize()
    return nc


def _get_module():
    global _CACHED
    if _CACHED is None:
        _CACHED = _build_module()
    return _CACHED


def kernel(x, k_table_v, k_table_h, v_table_v, v_table_h, _trace=False):
    global LAST_EXEC_NS
    from concourse.bass_utils import run_bass_kernel_spmd

    x = np.asarray(x, dtype=np.float32)
    iv, ih = _rel_indices()
    rel_k = np.asarray(k_table_v)[iv] + np.asarray(k_table_h)[ih]  # [N,N,D]
    rel_v = np.asarray(v_table_v)[iv] + np.asarray(v_table_h)[ih]  # [N,N,D]

    qkv = x.reshape(B, N, 3, H, D).transpose(2, 0, 3, 1, 4)  # [3,B,H,N,D]
    q, k, v = qkv[0], qkv[1], qkv[2]  # [B,H,N,D]

    rkT_host = np.ascontiguousarray(
        rel_k.transpose(2, 0, 1).astype(_bf16)
    )  # [D,N(i),N(j)]
    rv_host = np.ascontiguousarray(
        rel_v.transpose(1, 0, 2).astype(_bf16)
    )  # [N(j),N(i),D]

    in_maps = []
    for c in range(NCORES):
        qs = q[c * BSH : (c + 1) * BSH].reshape(BH, N, D)   # [BH,N,D]
        ks = k[c * BSH : (c + 1) * BSH].reshape(BH, N, D)
        vs = v[c * BSH : (c + 1) * BSH].reshape(BH, N, D)
        in_maps.append(
            {
                "qT": np.ascontiguousarray(qs.transpose(0, 2, 1)),  # [BH,D,N]
                "kT": np.ascontiguousarray(ks.transpose(0, 2, 1)),
                "vb": np.ascontiguousarray(vs.astype(_bf16)),
                "qTi": np.ascontiguousarray(
                    qs.transpose(2, 1, 0).astype(_bf16)
                ),  # [D,N,BH]
                "rkT": rkT_host,
                "rv": rv_host,
            }
        )

    nc = _get_module()
    res = run_bass_kernel_spmd(
        nc, in_maps, core_ids=list(range(NCORES)), trace=_trace
    )
    LAST_EXEC_NS = res.exec_time_ns
    global LAST_TRACE
    LAST_TRACE = res.instructions_and_trace
    outs = [res.results[c]["out"] for c in range(NCORES)]
    return np.concatenate(outs, axis=0).astype(np.float32)



# revision 4
# speedup vs baseline: 2.3412x; 2.3412x over previous
"""AttentionSuper (AutoFormer relative-position attention) on 8 trn2 cores.

Data-parallel over batch B=64 -> 8 batches/core (BH=80 fused (b,h) rows).
Per core, attention runs in TRANSPOSED score layout attnT[j, i] per (b,h):
  - value matmuls use attnT as lhsT directly, and softmax normalization
    comes free from a ones-column appended to v.
  - the rel-pos key bias is folded in multiplicatively:
      exp(scale*(s+b)) = exp(scale*s) * exp(scale*b)
    Phase A computes expB[j,i,bh] = exp(scale * q_i . rel_k[i,j]) for all
    (b,h) at once per i (PE matmul, ACT exp-evacuation from PSUM), and
    Phase B multiplies it into exp(scale*s) on the DVE (bf16 2x mode).
Everything is bf16 except PSUM accumulation (f32); scores are bounded
(|scale*s| < ~6) so softmax skips max-subtraction.

v2 rewrite targets instruction-count bottlenecks found in the v1 trace:
fp32 score matmuls (4x slower than bf16), 766 per-tensor dma_starts
(~600ns each on the Sync engine), and per-(bh,half) vector ops.
"""

import sys

import numpy as np

sys.path.insert(0, "/opt/trn_rl_repo")

import ml_dtypes  # noqa: E402

B, N, H, D = 64, 197, 10, 64
MAX_REL = 14
NCORES = 8
BSH = B // NCORES          # batches per core
BH = BSH * H               # 80 fused (batch, head) rows per core
P2 = N - 128               # 69
NP = N + 1                 # 198: padded i-stride (keeps bf16 rows 4B-aligned)
SCALE = D ** (-0.5)

_bf16 = ml_dtypes.bfloat16

LAST_EXEC_NS = None
LAST_TRACE = None
_CACHED = None


def _rel_indices():
    s = int(np.sqrt(N))
    r = np.arange(N)
    dist_v = r[None, :] // s - r[:, None] // s
    dist_h = r[None, :] % s - r[:, None] % s
    iv = np.clip(dist_v, -MAX_REL, MAX_REL) + MAX_REL + 1
    ih = np.clip(dist_h, -MAX_REL, MAX_REL) + MAX_REL + 1
    iv = np.pad(iv[1:, 1:], ((1, 0), (1, 0)), constant_values=0)
    ih = np.pad(ih[1:, 1:], ((1, 0), (1, 0)), constant_values=0)
    return iv, ih


def _build_module():
    import concourse.bass as bass  # noqa: F401
    import concourse.bacc as bacc
    import concourse.tile as tile
    from concourse import mybir

    f32 = mybir.dt.float32
    bf16 = mybir.dt.bfloat16
    Exp = mybir.ActivationFunctionType.Exp

    nc = bacc.Bacc()

    qT2 = nc.dram_tensor("qT2", [D, BH, N], bf16, kind="ExternalInput")
    kT2 = nc.dram_tensor("kT2", [D, BH, N], bf16, kind="ExternalInput")
    vb2 = nc.dram_tensor("vb2", [N, BH, D], bf16, kind="ExternalInput")
    qTi = nc.dram_tensor("qTi", [D, N, BH], bf16, kind="ExternalInput")
    rkT = nc.dram_tensor("rkT", [D, N, N], bf16, kind="ExternalInput")
    rv = nc.dram_tensor("rv", [N, N, D], bf16, kind="ExternalInput")
    out = nc.dram_tensor("out", [BSH, N, H * D], bf16, kind="ExternalOutput")
    o2d = nc.dram_tensor("o2d", [N, BH, D], bf16)  # internal: rel-v partial

    with tile.TileContext(nc) as tc:
        with tc.tile_pool(name="persist", bufs=1) as persist:
            # attnT[j, bh, i] (i-stride NP for alignment; col N is garbage)
            attnT_lo = persist.tile([128, BH, NP], bf16)
            attnT_hi = persist.tile([128, BH, NP], bf16)   # j in [128,197) on parts 0..68
            # raw O1 + denominator: [i, bh, 66] (col 64 = sum, col 65 = 0)
            o1sb_lo = persist.tile([128, BH, 66], bf16)
            o1sb_hi = persist.tile([128, BH, 66], bf16)
            recips = persist.tile([128, 2 * BH], f32)

            with tc.tile_pool(name="expb", bufs=1) as expbp:
                expB_lo = expbp.tile([128, BH, NP], bf16)
                expB_hi = expbp.tile([128, BH, NP], bf16)

                # ---- Phase A: expB[j, bh, i] = exp(scale * sum_d rel_k[i,j,d] q[bh,i,d])
                CH = 12
                with (
                    tc.tile_pool(name="pa", bufs=2) as pa,
                    tc.tile_pool(name="pap", bufs=2, space="PSUM") as pap,
                ):
                    for c0 in range(0, N, CH):
                        cn = min(CH, N - c0)
                        rk_t = pa.tile([D, CH, N], bf16, tag="rk")
                        nc.sync.dma_start(rk_t[:, :cn, :], rkT[:, c0 : c0 + cn, :])
                        qb_t = pa.tile([D, CH, BH], bf16, tag="qb")
                        nc.sync.dma_start(qb_t[:, :cn, :], qTi[:, c0 : c0 + cn, :])
                        ps_lo = pap.tile([128, 1024], f32, tag="pslo")
                        ps_hi = pap.tile([128, 1024], f32, tag="pshi")
                        for k in range(cn):
                            off = (k // 6) * 512 + (k % 6) * 80
                            nc.tensor.matmul(
                                ps_lo[:, off : off + 80],
                                rk_t[:, k, 0:128], qb_t[:, k, :],
                                start=True, stop=True,
                            )
                            nc.tensor.matmul(
                                ps_hi[0:P2, off : off + 80],
                                rk_t[:, k, 128:N], qb_t[:, k, :],
                                start=True, stop=True,
                            )
                        for ps, dst, npart in (
                            (ps_lo, expB_lo, 128), (ps_hi, expB_hi, P2)
                        ):
                            if cn == CH:
                                src = (
                                    ps[:]
                                    .rearrange("p (b r) -> p b r", b=2)[:, :, 0:480]
                                    .rearrange("p b (k e) -> p e b k", e=80)
                                )
                                d = dst[0:npart, :, c0 : c0 + cn].rearrange(
                                    "p h (b k) -> p h b k", b=2
                                )
                                nc.scalar.activation(
                                    d, src[0:npart], Exp, scale=SCALE
                                )
                            else:
                                src = ps[:, 0 : cn * 80].rearrange(
                                    "p (k e) -> p e k", e=80
                                )
                                nc.scalar.activation(
                                    dst[0:npart, :, c0 : c0 + cn],
                                    src[0:npart], Exp, scale=SCALE,
                                )

                # ---- Phase B1: per (b,h) pair: scoresT, exp, *expB, O1 = attnT.T @ [v|1]
                GB = 16
                with (
                    tc.tile_pool(name="pb", bufs=2) as pb,
                    tc.tile_pool(name="pbe", bufs=2) as pbe,
                    tc.tile_pool(name="pbp", bufs=2, space="PSUM") as pbp,
                ):
                    for g0 in range(0, BH, GB):
                        q2 = pb.tile([D, GB, N], bf16, tag="q2")
                        nc.sync.dma_start(q2[:], qT2[:, g0 : g0 + GB, :])
                        k2 = pb.tile([D, GB, N], bf16, tag="k2")
                        nc.sync.dma_start(k2[:], kT2[:, g0 : g0 + GB, :])
                        v2l = pb.tile([128, GB, 66], bf16, tag="v2l")
                        nc.sync.dma_start(
                            v2l[:, :, 0:64], vb2[0:128, g0 : g0 + GB, :]
                        )
                        nc.vector.memset(v2l[:, :, 64:65], 1.0)
                        nc.vector.memset(v2l[:, :, 65:66], 0.0)
                        v2h = pb.tile([128, GB, 66], bf16, tag="v2h")
                        nc.sync.dma_start(
                            v2h[0:P2, :, 0:64], vb2[128:N, g0 : g0 + GB, :]
                        )
                        nc.vector.memset(v2h[0:P2, :, 64:65], 1.0)
                        nc.vector.memset(v2h[0:P2, :, 65:66], 0.0)

                        for p0 in range(0, GB, 2):
                            bh = g0 + p0
                            ps_lo = pbp.tile([128, 2, NP], f32, tag="pslo")
                            ps_hi = pbp.tile([128, 2, NP], f32, tag="pshi")
                            for pp in range(2):
                                nc.tensor.matmul(
                                    ps_lo[:, pp, 0:197],
                                    k2[:, p0 + pp, 0:128], q2[:, p0 + pp, :],
                                    start=True, stop=True,
                                )
                                nc.tensor.matmul(
                                    ps_hi[0:P2, pp, 0:197],
                                    k2[:, p0 + pp, 128:N], q2[:, p0 + pp, :],
                                    start=True, stop=True,
                                )
                            es_lo = pbe.tile([128, 2, NP], bf16, tag="eslo")
                            nc.scalar.activation(
                                es_lo[:, :, 0:197], ps_lo[:, :, 0:197],
                                Exp, scale=SCALE,
                            )
                            es_hi = pbe.tile([128, 2, NP], bf16, tag="eshi")
                            nc.scalar.activation(
                                es_hi[0:P2, :, 0:197], ps_hi[0:P2, :, 0:197],
                                Exp, scale=SCALE,
                            )
                            nc.vector.tensor_mul(
                                attnT_lo[:, bh : bh + 2, :],
                                es_lo[:], expB_lo[:, bh : bh + 2, :],
                            )
                            nc.vector.tensor_mul(
                                attnT_hi[0:P2, bh : bh + 2, :],
                                es_hi[0:P2], expB_hi[0:P2, bh : bh + 2, :],
                            )

                            o1_lo = pbp.tile([128, 2, 66], f32, tag="o1lo")
                            o1_hi = pbp.tile([128, 2, 66], f32, tag="o1hi")
                            for pp in range(2):
                                nc.tensor.matmul(
                                    o1_lo[:, pp, :],
                                    attnT_lo[:, bh + pp, 0:128],
                                    v2l[:, p0 + pp, :],
                                    start=True, stop=False,
                                )
                                nc.tensor.matmul(
                                    o1_lo[:, pp, :],
                                    attnT_hi[0:P2, bh + pp, 0:128],
                                    v2h[0:P2, p0 + pp, :],
                                    start=False, stop=True,
                                )
                                nc.tensor.matmul(
                                    o1_hi[0:P2, pp, :],
                                    attnT_lo[:, bh + pp, 128:197],
                                    v2l[:, p0 + pp, :],
                                    start=True, stop=False,
                                )
                                nc.tensor.matmul(
                                    o1_hi[0:P2, pp, :],
                                    attnT_hi[0:P2, bh + pp, 128:197],
                                    v2h[0:P2, p0 + pp, :],
                                    start=False, stop=True,
                                )
                            nc.vector.tensor_copy(
                                o1sb_lo[:, bh : bh + 2, :], o1_lo[:]
                            )
                            nc.vector.tensor_copy(
                                o1sb_hi[0:P2, bh : bh + 2, :], o1_hi[0:P2]
                            )

                    nc.vector.reciprocal(
                        recips[:, 0:BH], o1sb_lo[:, :, 64:65].squeeze(2)
                    )
                    nc.vector.reciprocal(
                        recips[0:P2, BH : 2 * BH],
                        o1sb_hi[0:P2, :, 64:65].squeeze(2),
                    )

            # ---- Phase B2: O2[i, bh, d] = sum_j attnT[j, bh, i] rel_v[i, j, d]
            CH2 = 16
            with (
                tc.tile_pool(name="pc", bufs=2) as pc,
                tc.tile_pool(name="pcp", bufs=3, space="PSUM") as pcp,
            ):
                nb2 = 0
                for c0 in range(0, N, CH2):
                    cn = min(CH2, N - c0)
                    rvl = pc.tile([128, CH2, D], bf16, tag="rvl")
                    nc.sync.dma_start(rvl[:, :cn, :], rv[0:128, c0 : c0 + cn, :])
                    rvh = pc.tile([128, CH2, D], bf16, tag="rvh")
                    nc.sync.dma_start(rvh[0:P2, :cn, :], rv[128:N, c0 : c0 + cn, :])
                    o2s = pc.tile([BH, CH2, D], bf16, tag="o2s")
                    for hb in range(0, cn, 8):
                        gn = min(8, cn - hb)
                        o2 = pcp.tile([BH, 512], f32, tag="o2")
                        for k in range(gn):
                            i = c0 + hb + k
                            nc.tensor.matmul(
                                o2[:, k * 64 : (k + 1) * 64],
                                attnT_lo[:, :, i], rvl[:, hb + k, :],
                                start=True, stop=False,
                            )
                            nc.tensor.matmul(
                                o2[:, k * 64 : (k + 1) * 64],
                                attnT_hi[0:P2, :, i], rvh[0:P2, hb + k, :],
                                start=False, stop=True,
                            )
                        if nb2 % 2 == 0:
                            nc.vector.tensor_copy(
                                o2s[:, hb : hb + gn, :], o2[:, 0 : gn * 64]
                            )
                        else:
                            nc.scalar.copy(
                                o2s[:, hb : hb + gn, :], o2[:, 0 : gn * 64]
                            )
                        nb2 += 1
                    nc.sync.dma_start(
                        o2d[c0 : c0 + cn, :, :].transpose([1, 0, 2]),
                        o2s[:, :cn, :],
                    )

            tc.strict_bb_all_engine_barrier()

            # ---- Phase C: out[b, i, (h d)] = (O1raw + O2) * recip
            with tc.tile_pool(name="pd", bufs=2) as pd:
                for c0, cn, o1sb, rcol in (
                    (0, 128, o1sb_lo, 0), (128, P2, o1sb_hi, BH),
                ):
                    t2 = pd.tile([128, BH, D], bf16, tag="t2")
                    nc.sync.dma_start(t2[0:cn], o2d[c0 : c0 + cn, :, :])
                    addf = pd.tile([128, BH, D], bf16, tag="addf")
                    nc.vector.tensor_add(
                        addf[0:cn], t2[0:cn], o1sb[0:cn, :, 0:64]
                    )
                    res = pd.tile([128, BH, D], bf16, tag="res")
                    for bh in range(BH):
                        nc.vector.tensor_scalar_mul(
                            res[0:cn, bh, :], addf[0:cn, bh, :],
                            recips[0:cn, rcol + bh : rcol + bh + 1],
                        )
                    for b in range(BSH):
                        nc.sync.dma_start(
                            out[b, c0 : c0 + cn, :],
                            res[0:cn, b * H : (b + 1) * H, :],
                        )

    nc.finalize()
    return nc


def _get_module():
    global _CACHED
    if _CACHED is None:
        _CACHED = _build_module()
    return _CACHED


def kernel(x, k_table_v, k_table_h, v_table_v, v_table_h, _trace=False):
    global LAST_EXEC_NS, LAST_TRACE
    from concourse.bass_utils import run_bass_kernel_spmd

    x = np.asarray(x, dtype=np.float32)
    iv, ih = _rel_indices()
    rel_k = np.asarray(k_table_v)[iv] + np.asarray(k_table_h)[ih]  # [N,N,D]
    rel_v = np.asarray(v_table_v)[iv] + np.asarray(v_table_h)[ih]  # [N,N,D]

    qkv = x.reshape(B, N, 3, H, D).transpose(2, 0, 3, 1, 4)  # [3,B,H,N,D]
    q, k, v = qkv[0], qkv[1], qkv[2]  # [B,H,N,D]

    rkT_host = np.ascontiguousarray(
        rel_k.transpose(2, 0, 1).astype(_bf16)
    )  # [D,N(i),N(j)]
    rv_host = np.ascontiguousarray(
        rel_v.transpose(1, 0, 2).astype(_bf16)
    )  # [N(j),N(i),D]

    in_maps = []
    for c in range(NCORES):
        qs = q[c * BSH : (c + 1) * BSH].reshape(BH, N, D)   # [BH,N,D]
        ks = k[c * BSH : (c + 1) * BSH].reshape(BH, N, D)
        vs = v[c * BSH : (c + 1) * BSH].reshape(BH, N, D)
        in_maps.append(
            {
                "qT2": np.ascontiguousarray(
                    qs.transpose(2, 0, 1).astype(_bf16)
                ),  # [D,BH,N]
                "kT2": np.ascontiguousarray(
                    ks.transpose(2, 0, 1).astype(_bf16)
                ),
                "vb2": np.ascontiguousarray(
                    vs.transpose(1, 0, 2).astype(_bf16)
                ),  # [N,BH,D]
                "qTi": np.ascontiguousarray(
                    qs.transpose(2, 1, 0).astype(_bf16)
                ),  # [D,N,BH]
                "rkT": rkT_host,
                "rv": rv_host,
            }
        )

    nc = _get_module()
    res = run_bass_kernel_spmd(
        nc, in_maps, core_ids=list(range(NCORES)), trace=_trace
    )
    LAST_EXEC_NS = res.exec_time_ns
    LAST_TRACE = res.instructions_and_trace
    outs = [res.results[c]["out"].astype(np.float32) for c in range(NCORES)]
    return np.concatenate(outs, axis=0)
